# revision 1
# baseline (speedup 1.0000x reference)
"""Trainium2 Bass kernel for nn_Decoder_14894946583396 (dense_mlp).

Reference computation:
    sized = broadcast(representation[B,1,R] -> [B,S,R])   (ones @ rep)
    h     = relu(sized @ W1^T + b1)                       [B,S,HID]
    out   = h @ W2^T + b2                                 [B,S,OUT]

Because every position s within batch b receives the identical input row
representation[b], the MLP output row is identical for all S positions:
    row[b] = relu(rep[b] @ W1^T + b1) @ W2^T + b2         [B,OUT]
    out[b, s, :] = row[b]  for all s

The kernel computes the tiny per-batch MLP on the TensorEngine (fp32,
bit-exact vs the f32 reference) and broadcast-writes each row across S
with wide SBUF->DRAM DMAs. Data-parallel across 8 NeuronCores: 4 batches
per core, replicated weights.

Device pipeline per core:
  1. Four input DMAs: pk1a = {x^T, I4} (tiny, HWDGE lane 0 — it gates
     compute), prow = {b1, ones, b2} single row, w1 = W1^T, w2 = W2^T
     (all three on SWDGE lanes, streaming under the warmup).
  2. ~10 us of dummy matmuls on zeros warm the PE HAM clock gate
     (1.2 -> 2.4 GHz) while weights stream in.
  3. L1: H[m,h] = x @ W1^T via 8 accumulating matmuls with the tiny x^T
     chunk as stationary (cheap LDWEIGHTS), bias folded in as a K=1
     ones-matmul, relu on ScalarE.
  4. H -> H^T via 4 PE transposes (stationary operand for L2).
  5. L2: Y[m,o] = H @ W2^T + b2, 10 matmuls into 2 PSUM banks.
  6. Y rows moved to partition-0 tiles by tiny SBUF->SBUF DMAs (matmul
     operands must start at partition 0/32/64).
  7. Broadcast: K=1 matmul with a ones row as stationary -> [128,512]
     PSUM tiles where every partition holds row[b]; one PSUM->SBUF copy
     per half, then wide SBUF->SBUF replication copies (one writer
     engine per tile).
  8. 8 output DMAs of 2 MiB each on the 8 fresh HWDGE lanes.

Single-sync-wait discipline (this walrus rejects 2+ waits on any
instruction): inputs are packed so every consumer sees one DMA
semaphore; SWDGE lanes carry everything but pk1a and the outputs so no
HWDGE output trigger reuses a lane whose data wait is unobserved;
artificial add_dep_helper edges pre-observe upcoming DMA lanes on
instructions that have a free wait slot; and a chain of 1-wait SP nops
before the TileContext exit drain leaves the drain with nothing to wait
on.
"""

import sys

import numpy as np

if "/opt/trn_rl_repo" not in sys.path:
    sys.path.insert(0, "/opt/trn_rl_repo")

B, S, R = 32, 1024, 1024
HID, OUT = 512, 1024
N_CORES = 8
BPC = B // N_CORES  # batches per core

RC = R // 128  # layer-1 contraction chunks
HC = HID // 128  # layer-2 contraction chunks
OC = OUT // 512  # 512-wide output column chunks

# pk1a columns: [p, rc*BPC + m] = rep[m, rc*128+p], then a 4x4 identity,
# then 4 selector-broadcast blocks: [k, SELOFF + b*128 + m] = (k == b)
XTOFF = 0
I4OFF = XTOFF + RC * BPC
SELOFF = I4OFF + BPC
PK1AW = SELOFF + BPC * 128
# prow columns (single partition row)
B1OFF = 0
ONOFF = B1OFF + HID
B2OFF = ONOFF + 128
PROWW = B2OFF + OUT
# w1: [p, rc*HID + h] = W1[h, rc*128+p];  w2: [p, hc*OUT + o] = W2[o, hc*128+p]

N_COPIES = 4  # row copies along the free dim of each broadcast tile
S_PER_DMA = 128 * N_COPIES  # s-positions covered per output DMA
N_DMAS = S // S_PER_DMA  # output DMAs per batch
N_WARMUP = 8

_CACHED_NC = None


def _build_nc():
    import concourse.bass as bass
    import concourse.mybir as mybir
    from concourse.tile import TileContext, add_dep_helper

    f32 = mybir.dt.float32
    relu = mybir.ActivationFunctionType.Relu
    fcopy = mybir.ActivationFunctionType.Copy
    nc = bass.Bass()

    pk1a = nc.dram_tensor("pk1a", [128, PK1AW], f32, kind="ExternalInput")
    prow = nc.dram_tensor("prow", [1, PROWW], f32, kind="ExternalInput")
    w1 = nc.dram_tensor("w1", [128, RC * HID], f32, kind="ExternalInput")
    w2 = nc.dram_tensor("w2", [128, HC * OUT], f32, kind="ExternalInput")
    out = nc.dram_tensor("out", [BPC, S, OUT], f32, kind="ExternalOutput")

    with TileContext(nc) as tc:
        with (
            tc.tile_pool(name="const", bufs=1) as cpool,
            tc.tile_pool(name="psum_s", bufs=1, space="PSUM") as pp_s,
            tc.tile_pool(name="psum_y", bufs=2, space="PSUM") as pp_y,
            tc.tile_pool(name="psum_t", bufs=1, space="PSUM") as pp_t,
            tc.tile_pool(name="psum_bc", bufs=4, space="PSUM") as pp_bc,
        ):
            p1a = cpool.tile([128, PK1AW], f32, tag="pk1a")
            nc.sync.dma_start(out=p1a[:, :], in_=pk1a[:, :])
            prow_sb = cpool.tile([1, PROWW], f32, tag="prow")
            dma_prow = nc.gpsimd.dma_start(out=prow_sb[0:1, :], in_=prow[0:1, :])
            w1_sb = cpool.tile([128, RC * HID], f32, tag="w1")
            dma_w1 = nc.gpsimd.dma_start(out=w1_sb[:, :], in_=w1[:, :])
            w2_sb = cpool.tile([128, HC * OUT], f32, tag="w2")
            dma_w2 = nc.gpsimd.dma_start(out=w2_sb[:, :], in_=w2[:, :])

            # ---- PE warmup on zeros; shares L1's PSUM tile (a slot handoff
            # would emit a non-elidable same-engine wait) -------------------
            wm_sb = cpool.tile([128, 512], f32, tag="wm")
            nc.vector.memset(wm_sb[:, :], 0.0)
            ph_full = pp_s.tile([128, HID], f32, tag="s")
            for k in range(N_WARMUP):
                wmm = nc.tensor.matmul(
                    ph_full[:, :],
                    lhsT=wm_sb[:, 0:128],
                    rhs=wm_sb[:, :],
                    start=True,
                    stop=True,
                )
            # the last warmup matmul observes w1's lane so L1's first matmul
            # only needs the pk1a wait
            add_dep_helper(wmm.ins, dma_w1.ins, sync=True, reason="observe w1")

            # ---- L1: H[m, h] = x @ W1^T + b1, relu -------------------------
            ph = ph_full[0:BPC, :]
            for rc in range(RC):
                mm = nc.tensor.matmul(
                    ph[:, :],
                    lhsT=p1a[:, XTOFF + rc * BPC : XTOFF + (rc + 1) * BPC],
                    rhs=w1_sb[:, rc * HID : rc * HID + HID],
                    start=(rc == 0),
                    stop=False,
                )
            # rc=7 has a free wait slot: pre-observe w2's lane for L2
            add_dep_helper(mm.ins, dma_w2.ins, sync=True, reason="observe w2")
            nc.tensor.matmul(
                ph[:, :],
                lhsT=prow_sb[0:1, ONOFF : ONOFF + BPC],
                rhs=prow_sb[0:1, B1OFF : B1OFF + HID],
                start=False,
                stop=True,
            )
            h_sb = cpool.tile([BPC, HID], f32, tag="h")
            nc.scalar.activation(h_sb[:, :], ph[:, :], relu)

            # ---- H -> H^T (stationary operand for L2) ----------------------
            ht_sb = cpool.tile([128, HC * BPC], f32, tag="ht")
            for hc in range(HC):
                pt = pp_t.tile([128, BPC], f32, tag="t")
                nc.tensor.transpose(
                    pt[:, :],
                    h_sb[0:BPC, hc * 128 : (hc + 1) * 128],
                    p1a[0:BPC, I4OFF : I4OFF + BPC],
                )
                nc.scalar.activation(
                    ht_sb[:, hc * BPC : (hc + 1) * BPC], pt[:, :], fcopy
                )

            # ---- L2: Y[m, o] = H @ W2^T + b2 -------------------------------
            # per-oc Y tiles so the broadcast of the first half can start
            # while the second half's matmuls still run
            y_halves = []
            for oc in range(OC):
                py = pp_y.tile([BPC, 512], f32, tag="y")
                for hc in range(HC):
                    nc.tensor.matmul(
                        py[:, :],
                        lhsT=ht_sb[:, hc * BPC : (hc + 1) * BPC],
                        rhs=w2_sb[:, hc * OUT + oc * 512 : hc * OUT + oc * 512 + 512],
                        start=(hc == 0),
                        stop=False,
                    )
                nc.tensor.matmul(
                    py[:, :],
                    lhsT=prow_sb[0:1, ONOFF : ONOFF + BPC],
                    rhs=prow_sb[0:1, B2OFF + oc * 512 : B2OFF + (oc + 1) * 512],
                    start=False,
                    stop=True,
                )
                yh = cpool.tile([BPC, 512], f32, tag=f"yh{oc}")
                nc.vector.tensor_copy(yh[:, :], py[:, :])
                y_halves.append(yh)

            # ---- broadcast rows across partitions, replicate, store --------
            # A K=4 selector matmul (lhsT = e_b outer ones, host-packed)
            # extracts row b of Y AND replicates it across all 128 output
            # partitions in one PE op — both operands at base partition 0.
            out_dmas = []
            for b in range(BPC):
                yt = cpool.tile([128, N_COPIES * OUT], f32, tag=f"yt{b}")
                copy_eng = "dve" if b % 2 == 0 else "act"
                for oc in range(OC):
                    pb = pp_bc.tile([128, 512], f32, tag="bc")
                    mm = nc.tensor.matmul(
                        pb[:, :],
                        lhsT=p1a[0:BPC, SELOFF + b * 128 : SELOFF + (b + 1) * 128],
                        rhs=y_halves[oc][0:BPC, :],
                        start=True,
                        stop=True,
                    )
                    last_mm = mm
                    # PSUM -> SBUF once per oc half...
                    dst = yt[:, oc * 512 : (oc + 1) * 512]
                    if copy_eng == "dve":
                        last_dve = nc.vector.tensor_copy(dst, pb[:, :])
                    else:
                        last_act = nc.scalar.activation(dst, pb[:, :], fcopy)
                # ...then replicate with wide SBUF->SBUF copies (2x f32 mode)
                for c in range(1, N_COPIES):
                    dst = yt[:, c * OUT : (c + 1) * OUT]
                    if copy_eng == "dve":
                        last_dve = nc.vector.tensor_copy(dst, yt[:, 0:OUT])
                    else:
                        last_act = nc.scalar.activation(dst, yt[:, 0:OUT], fcopy)
                # each DMA writes S_PER_DMA consecutive s rows (all identical)
                for j in range(N_DMAS):
                    d = nc.sync.dma_start(
                        out=out[b, j * S_PER_DMA : (j + 1) * S_PER_DMA, :].rearrange(
                            "(p c) o -> p c o", c=N_COPIES
                        ),
                        in_=yt[:, :].rearrange("p (c o) -> p c o", o=OUT),
                    )
                    out_dmas.append(d)

            # The kernel-tail drain waits on every proc's final tick, but this
            # walrus allows at most ONE sync wait per instruction. Chain SP
            # nops, one dependency each, so SP's vector clock observes the
            # final tick of every DMA lane and engine before the drain.
            tail = out_dmas + [dma_prow, dma_w1, dma_w2, last_mm, last_act, last_dve]
            for d in tail:
                n = nc.sync.nop(nofuse=True)
                add_dep_helper(
                    n.ins, d.ins, sync=True, reason="observe final ticks pre-drain"
                )

    return nc


def _get_nc():
    global _CACHED_NC
    if _CACHED_NC is None:
        _CACHED_NC = _build_nc()
    return _CACHED_NC


def _prep_in_maps(representation, W1, b1, W2, b2):
    rep = np.asarray(representation, dtype=np.float32).reshape(B, R)
    w1 = np.asarray(W1, dtype=np.float32)
    w2 = np.asarray(W2, dtype=np.float32)
    b1 = np.asarray(b1, dtype=np.float32)
    b2 = np.asarray(b2, dtype=np.float32)

    w1p = np.ascontiguousarray(
        w1.T.reshape(RC, 128, HID).transpose(1, 0, 2).reshape(128, RC * HID)
    )
    w2p = np.ascontiguousarray(
        w2.T.reshape(HC, 128, OUT).transpose(1, 0, 2).reshape(128, HC * OUT)
    )
    prow = np.zeros((1, PROWW), dtype=np.float32)
    prow[0, B1OFF : B1OFF + HID] = b1
    prow[0, ONOFF : ONOFF + 128] = 1.0
    prow[0, B2OFF : B2OFF + OUT] = b2

    in_maps = []
    for c in range(N_CORES):
        xt = rep[c * BPC : (c + 1) * BPC].T  # [R, BPC]
        pk1a = np.zeros((128, PK1AW), dtype=np.float32)
        pk1a[:, XTOFF : XTOFF + RC * BPC] = (
            xt.reshape(RC, 128, BPC).transpose(1, 0, 2).reshape(128, RC * BPC)
        )
        pk1a[0:BPC, I4OFF : I4OFF + BPC] = np.eye(BPC, dtype=np.float32)
        for b in range(BPC):
            pk1a[b, SELOFF + b * 128 : SELOFF + (b + 1) * 128] = 1.0
        in_maps.append({"pk1a": pk1a, "prow": prow, "w1": w1p, "w2": w2p})
    return in_maps


def run_sharded(representation, W1, b1, W2, b2, **run_kwargs):
    """Compile+run on 8 cores; returns (full_output, BassKernelResults)."""
    from concourse.bass_utils import run_bass_kernel_spmd

    nc = _get_nc()
    in_maps = _prep_in_maps(representation, W1, b1, W2, b2)
    res = run_bass_kernel_spmd(nc, in_maps, core_ids=list(range(N_CORES)), **run_kwargs)
    full = np.concatenate([r["out"] for r in res.results], axis=0)
    return full, res


def kernel(representation, size_matrix=None, W1=None, b1=None, W2=None, b2=None):
    # size_matrix only contributes its shape in the reference (ones_like);
    # its values are unused.
    full, _ = run_sharded(representation, W1, b1, W2, b2)
    return full



# revision 8
# speedup vs baseline: 1.2245x; 1.2245x over previous
"""Trainium2 Bass kernel for nn_Decoder_14894946583396 (dense_mlp).

Reference computation:
    sized = broadcast(representation[B,1,R] -> [B,S,R])   (ones @ rep)
    h     = relu(sized @ W1^T + b1)                       [B,S,HID]
    out   = h @ W2^T + b2                                 [B,S,OUT]

Because every position s within batch b receives the identical input row
representation[b], the MLP output row is identical for all S positions:
    row[b] = relu(rep[b] @ W1^T + b1) @ W2^T + b2         [B,OUT]
    out[b, s, :] = row[b]  for all s

Data-parallel across 8 NeuronCores: 4 batches per core, replicated
weights. The per-core kernel computes the tiny MLP in fp16 on the
TensorEngine (1 cycle/row vs fp32's 4) and broadcast-writes each row
across S with stride-0-source SBUF->DRAM DMAs.

Device pipeline per core:
  1. Parallel input DMAs on three queues: pk1 = {x^T, I4, selectors,
     b1, b2 halves} (271 KB, sync HWDGE), W1^T fp16 in two 512 KB
     chunks (scalar HWDGE), W2^T fp16 in two per-output-half chunks
     (gpsimd SWDGE).
  2. 5 fp16 warmup matmuls on zeros nudge the PE HAM clock ramp while
     w1's first chunk streams in; the last one observes w1h0's lane.
  3. L1: H[m,h] = x @ W1^T via 8 accumulating fp16 matmuls (x^T chunk
     stationary) + a K=1 ones-matmul folding in b1; relu on ScalarE.
  4. H -> H^T via 4 PE transposes (f32), cast to fp16 on DVE.
  5. L2 per 512-col half: Y[m,o] = H @ W2^T + b2, 5 fp16 matmuls; PSUM
     -> SBUF fp16 cast on DVE.
  6. Broadcast per batch/half: K=4 selector matmul replicates row b
     across 128 partitions; PSUM -> SBUF f32 copy (DVE even b / ACT
     odd b).
  7. One 4 MiB DMA per batch: out[b] viewed [p=128, c=8, o=1024] fed
     from the [128,1024] broadcast tile with a stride-0 c dim (each
     partition's 4 KB row is read 8x). Even b on sync, odd b on
     scalar, so the two HWDGE rings interleave.

Single-sync-wait discipline (this walrus rejects 2+ waits on any
instruction): the last warmup matmul pre-observes w1h0's lane so L1's
first matmul only waits pk1; transposes 2/3 pre-observe w2's two lane
ticks so L2's first matmuls only wait the DVE ht-cast tick; odd-batch
broadcast copies run on ACT so the scalar-ring output triggers need no
cross-engine wait; and a chain of 1-wait SP nops before the
TileContext exit drain leaves the drain with nothing to wait on.
"""

import sys

import numpy as np

if "/opt/trn_rl_repo" not in sys.path:
    sys.path.insert(0, "/opt/trn_rl_repo")

B, S, R = 32, 1024, 1024
HID, OUT = 512, 1024
N_CORES = 8
BPC = B // N_CORES  # batches per core

RC = R // 128  # layer-1 contraction chunks
HC = HID // 128  # layer-2 contraction chunks
OC = OUT // 512  # 512-wide output column chunks
SC = S // 128  # broadcast repeats per output DMA

# pk1 columns (fp16): x^T chunks, 4x4 identity, 4 selector blocks, and
# bias rows overlaid on spare partitions (matmul operands may start at
# partition 0/32/64).
XTOFF = 0
I4OFF = XTOFF + RC * BPC  # 32
SELOFF = I4OFF + BPC  # 36
B2H0OFF = SELOFF + BPC * 128  # 548
B2H1OFF = B2H0OFF + 512  # 1060
PK1W = B2H1OFF + 512  # 1572
# matmul lhsT/rhs must share a base partition: all K=1 bias rows live on
# partition 32 (ones at cols I4OFF.., b1 at SELOFF.., b2 halves after)
ONESP = 32
B1P = 32
B2P = 32

N_WARMUP = 5

_CACHED_NC = None


def _build_nc():
    import concourse.bass as bass
    import concourse.mybir as mybir
    from concourse.tile import TileContext, add_dep_helper

    f32 = mybir.dt.float32
    f16 = mybir.dt.float16
    relu = mybir.ActivationFunctionType.Relu
    nc = bass.Bass()

    pk1 = nc.dram_tensor("pk1", [128, PK1W], f16, kind="ExternalInput")
    w1 = nc.dram_tensor("w1", [128, RC * HID], f16, kind="ExternalInput")
    w2 = nc.dram_tensor("w2", [128, OC * HC * 512], f16, kind="ExternalInput")
    out = nc.dram_tensor("out", [BPC, S, OUT], f32, kind="ExternalOutput")

    with TileContext(nc) as tc:
        with (
            tc.tile_pool(name="const", bufs=1) as cpool,
            tc.tile_pool(name="psum_s", bufs=1, space="PSUM") as pp_s,
            tc.tile_pool(name="psum_y", bufs=2, space="PSUM") as pp_y,
            tc.tile_pool(name="psum_t", bufs=1, space="PSUM") as pp_t,
            tc.tile_pool(name="psum_bc", bufs=4, space="PSUM") as pp_bc,
        ):
            p1 = cpool.tile([128, PK1W], f16, tag="pk1")
            dma_pk1 = nc.sync.dma_start(out=p1[:, :], in_=pk1[:, :])
            w1_sb = cpool.tile([128, RC * HID], f16, tag="w1")
            half = RC * HID // 2
            dma_w1a = nc.scalar.dma_start(out=w1_sb[:, 0:half], in_=w1[:, 0:half])
            dma_w1b = nc.scalar.dma_start(out=w1_sb[:, half:], in_=w1[:, half:])
            w2_sb = cpool.tile([128, OC * HC * 512], f16, tag="w2")
            dma_w2a = nc.gpsimd.dma_start(
                out=w2_sb[:, 0 : HC * 512], in_=w2[:, 0 : HC * 512]
            )
            dma_w2b = nc.gpsimd.dma_start(
                out=w2_sb[:, HC * 512 :], in_=w2[:, HC * 512 :]
            )

            # ---- PE warmup on zeros; shares L1's PSUM tile (a slot handoff
            # would emit a non-elidable same-engine wait) -------------------
            wm_sb = cpool.tile([128, 512], f16, tag="wm")
            nc.vector.memset(wm_sb[:, :], 0.0)
            ph_full = pp_s.tile([128, HID], f32, tag="s")
            for k in range(N_WARMUP):
                wmm = nc.tensor.matmul(
                    ph_full[:, :],
                    lhsT=wm_sb[:, 0:128],
                    rhs=wm_sb[:, :],
                    start=True,
                    stop=True,
                )
            # the last warmup matmul observes w1h0's lane so L1's first
            # matmul only needs the pk1 wait
            add_dep_helper(wmm.ins, dma_w1a.ins, sync=True, reason="observe w1h0")

            # ---- L1: H[m, h] = x @ W1^T + b1, relu -------------------------
            ph = ph_full[0:BPC, :]
            for rc in range(RC):
                nc.tensor.matmul(
                    ph[:, :],
                    lhsT=p1[:, XTOFF + rc * BPC : XTOFF + (rc + 1) * BPC],
                    rhs=w1_sb[:, rc * HID : rc * HID + HID],
                    start=(rc == 0),
                    stop=False,
                )
            bmm = nc.tensor.matmul(
                ph[:, :],
                lhsT=p1[ONESP : ONESP + 1, I4OFF : I4OFF + BPC],
                rhs=p1[B1P : B1P + 1, SELOFF : SELOFF + HID],
                start=False,
                stop=True,
            )
            # free wait slot on the bias matmul: observing w2b's (later) tick
            # on the gpsimd lane also covers w2a, so L2's first matmuls only
            # need the DVE ht-cast tick
            add_dep_helper(bmm.ins, dma_w2b.ins, sync=True, reason="observe w2")
            h_sb = cpool.tile([BPC, HID], f16, tag="h")
            nc.scalar.activation(h_sb[:, :], ph[:, :], relu)

            # ---- H -> H^T (fp16, stationary operand for L2) ----------------
            ht_sb = cpool.tile([128, HC * BPC], f16, tag="ht")
            for hc in range(HC):
                pt = pp_t.tile([128, BPC], f16, tag="t")
                nc.tensor.transpose(
                    pt[:, :],
                    h_sb[0:BPC, hc * 128 : (hc + 1) * 128],
                    p1[0:BPC, I4OFF : I4OFF + BPC],
                )
                nc.vector.tensor_copy(ht_sb[:, hc * BPC : (hc + 1) * BPC], pt[:, :])

            # ---- L2: Y[m, o] = H @ W2^T + b2, fp16 y rows ------------------
            y_halves = []
            for oc in range(OC):
                py = pp_y.tile([BPC, 512], f32, tag="y")
                for hc in range(HC):
                    nc.tensor.matmul(
                        py[:, :],
                        lhsT=ht_sb[:, hc * BPC : (hc + 1) * BPC],
                        rhs=w2_sb[:, oc * HC * 512 + hc * 512 : oc * HC * 512 + (hc + 1) * 512],
                        start=(hc == 0),
                        stop=False,
                    )
                b2c = B2H0OFF if oc == 0 else B2H1OFF
                nc.tensor.matmul(
                    py[:, :],
                    lhsT=p1[ONESP : ONESP + 1, I4OFF : I4OFF + BPC],
                    rhs=p1[B2P : B2P + 1, b2c : b2c + 512],
                    start=False,
                    stop=True,
                )
                yh = cpool.tile([BPC, 512], f16, tag=f"yh{oc}")
                nc.vector.tensor_copy(yh[:, :], py[:, :])
                y_halves.append(yh)

            # ---- broadcast rows across partitions, store -------------------
            # A K=4 selector matmul (lhsT = e_b outer ones, host-packed)
            # extracts row b of Y AND replicates it across all 128 output
            # partitions in one PE op. One 4 MiB DMA per batch reads the
            # [128,1024] tile 8x via a stride-0 dim.
            out_dmas = []
            last_act = None
            last_dve = None
            for b in range(BPC):
                ybc = cpool.tile([128, OUT], f32, tag=f"ybc{b}")
                for oc in range(OC):
                    pb = pp_bc.tile([128, 512], f32, tag="bc")
                    last_mm = nc.tensor.matmul(
                        pb[:, :],
                        lhsT=p1[0:BPC, SELOFF + b * 128 : SELOFF + (b + 1) * 128],
                        rhs=y_halves[oc][0:BPC, :],
                        start=True,
                        stop=True,
                    )
                    dst = ybc[:, oc * 512 : (oc + 1) * 512]
                    if b % 2 == 0:
                        last_dve = nc.vector.tensor_copy(dst, pb[:, :])
                    else:
                        last_act = nc.scalar.activation(
                            dst, pb[:, :], mybir.ActivationFunctionType.Copy
                        )
                dma_eng = nc.sync if b % 2 == 0 else nc.scalar
                d = dma_eng.dma_start(
                    out=out[b].rearrange("(c p) o -> p c o", c=SC),
                    in_=ybc[:, :].unsqueeze(1).broadcast_to((128, SC, OUT)),
                )
                out_dmas.append(d)

            # The kernel-tail drain waits on every proc's final tick, but this
            # walrus allows at most ONE sync wait per instruction. Chain SP
            # nops, one dependency each, so SP's vector clock observes the
            # final tick of every DMA lane and engine before the drain.
            tail = out_dmas + [
                dma_pk1, dma_w1a, dma_w1b, dma_w2a, dma_w2b,
                last_mm, last_act, last_dve,
            ]
            for d in tail:
                n = nc.sync.nop(nofuse=True)
                add_dep_helper(
                    n.ins, d.ins, sync=True, reason="observe final ticks pre-drain"
                )

    return nc


def _get_nc():
    global _CACHED_NC
    if _CACHED_NC is None:
        _CACHED_NC = _build_nc()
    return _CACHED_NC


def _prep_in_maps(representation, W1, b1, W2, b2):
    rep = np.asarray(representation, dtype=np.float32).reshape(B, R)
    w1 = np.asarray(W1, dtype=np.float32)
    w2 = np.asarray(W2, dtype=np.float32)
    b1 = np.asarray(b1, dtype=np.float32)
    b2 = np.asarray(b2, dtype=np.float32)

    # w1p[p, rc*HID + h] = W1[h, rc*128 + p]
    w1p = np.ascontiguousarray(
        w1.T.reshape(RC, 128, HID).transpose(1, 0, 2).reshape(128, RC * HID)
    ).astype(np.float16)
    # w2p[p, oc*HC*512 + hc*512 + j] = W2[oc*512 + j, hc*128 + p]
    w2p = (
        w2.reshape(OC, 512, HC, 128)
        .transpose(3, 0, 2, 1)
        .reshape(128, OC * HC * 512)
    )
    w2p = np.ascontiguousarray(w2p).astype(np.float16)

    in_maps = []
    for c in range(N_CORES):
        xt = rep[c * BPC : (c + 1) * BPC].T  # [R, BPC]
        pk1 = np.zeros((128, PK1W), dtype=np.float16)
        pk1[:, XTOFF : XTOFF + RC * BPC] = (
            xt.reshape(RC, 128, BPC).transpose(1, 0, 2).reshape(128, RC * BPC)
        ).astype(np.float16)
        pk1[0:BPC, I4OFF : I4OFF + BPC] = np.eye(BPC, dtype=np.float16)
        pk1[ONESP, I4OFF : I4OFF + BPC] = 1.0
        for b in range(BPC):
            pk1[b, SELOFF + b * 128 : SELOFF + (b + 1) * 128] = 1.0
        pk1[B1P, SELOFF : SELOFF + HID] = b1.astype(np.float16)
        pk1[B2P, B2H0OFF : B2H0OFF + 512] = b2[0:512].astype(np.float16)
        pk1[B2P, B2H1OFF : B2H1OFF + 512] = b2[512:1024].astype(np.float16)
        in_maps.append({"pk1": pk1, "w1": w1p, "w2": w2p})
    return in_maps


def run_sharded(representation, W1, b1, W2, b2, **run_kwargs):
    """Compile+run on 8 cores; returns (full_output, BassKernelResults)."""
    from concourse.bass_utils import run_bass_kernel_spmd

    nc = _get_nc()
    in_maps = _prep_in_maps(representation, W1, b1, W2, b2)
    res = run_bass_kernel_spmd(nc, in_maps, core_ids=list(range(N_CORES)), **run_kwargs)
    full = np.concatenate([r["out"] for r in res.results], axis=0)
    return full, res


def kernel(representation, size_matrix=None, W1=None, b1=None, W2=None, b2=None):
    # size_matrix only contributes its shape in the reference (ones_like);
    # its values are unused.
    full, _ = run_sharded(representation, W1, b1, W2, b2)
    return full


# revision 11
# speedup vs baseline: 1.3999x; 1.1433x over previous
"""Trainium2 Bass kernel for nn_Decoder_14894946583396 (dense_mlp).

Reference computation:
    sized = broadcast(representation[B,1,R] -> [B,S,R])   (ones @ rep)
    h     = relu(sized @ W1^T + b1)                       [B,S,HID]
    out   = h @ W2^T + b2                                 [B,S,OUT]

Because every position s within batch b receives the identical input row
representation[b], the MLP output row is identical for all S positions:
    row[b] = relu(rep[b] @ W1^T + b1) @ W2^T + b2         [B,OUT]
    out[b, s, :] = row[b]  for all s

Data-parallel across 8 NeuronCores: 4 batches per core, replicated
weights. The per-core kernel computes the tiny MLP in fp16 on the
TensorEngine (1 cycle/row vs fp32's 4) and broadcast-writes each row
across S with stride-0-source SBUF->DRAM DMAs.

Device pipeline per core:
  1. ALL inputs live in one packed fp16 DRAM tensor, streamed over the
     sync HWDGE ring as 5 chunked DMAs in consumption order:
       A: x^T + I4 + ones rows + W1h0   (so L1 starts on arrival)
       B: W1h1 + bias block (b1@p32, b2h0@p64, b2h1@p96)
       C0/C1: W2 per output half
       D: selector blocks
     A single ring avoids the cross-queue SDMA thrash that halved
     aggregate read bandwidth when inputs were spread over 3 queues.
  2. No warmup: L1's own matmul stream is the HAM-ramp activity.
  3. L1: H[m,h] = x @ W1^T via 8 accumulating fp16 matmuls (x^T chunk
     stationary) + a K=1 ones-matmul folding in b1; relu casts to fp16
     on ScalarE.
  4. H -> H^T via 4 fp16 PE transposes, copied to SBUF on DVE.
  5. L2 per 512-col half: Y = H @ W2^T + b2 (5 fp16 matmuls + K=1 bias
     matmul vs ones@p64/p96); PSUM -> SBUF fp16 cast (DVE half 0, ACT
     half 1).
  6. Broadcast per batch/half: K=4 selector matmul replicates row b
     across 128 partitions; PSUM -> SBUF f32 copy (DVE even b, ACT
     odd b).
  7. One 4 MiB DMA per batch: out[b] viewed [p=128, c=8, o=1024] fed
     from the [128,1024] broadcast tile with a stride-0 c dim (each
     partition's 4 KB row is read 8x). Even b on the sync ring, odd b
     on the scalar ring.

The Tile layer auto-inserts single-wait sync NOPs where an instruction
would need 2+ semaphore waits; the explicit nop chain before the
TileContext exit keeps the final drain itself at <=1 wait.
"""

import sys

import numpy as np

if "/opt/trn_rl_repo" not in sys.path:
    sys.path.insert(0, "/opt/trn_rl_repo")

B, S, R = 32, 1024, 1024
HID, OUT = 512, 1024
N_CORES = 8
BPC = B // N_CORES  # batches per core

RC = R // 128  # layer-1 contraction chunks
HC = HID // 128  # layer-2 contraction chunks
OC = OUT // 512  # 512-wide output column chunks
SC = S // 128  # broadcast repeats per output DMA

# pin columns (fp16), in DMA-chunk order:
#   A: x^T (32) | I4+ones (4) | W1h0 (2048)
#   B: W1h1 (2048) | bias block (512: b1@p32, b2h0@p64, b2h1@p96)
#   C: W2 oc0 (2048) | W2 oc1 (2048)
#   D: selector blocks (512, rows 0..3)
XTOFF = 0
I4OFF = XTOFF + RC * BPC  # 32
W1OFF = I4OFF + BPC  # 36
BIASOFF = W1OFF + RC * HID  # 4132, first 4 cols: ones row at p0
BVAL = BIASOFF + 4  # bias values: b1@p32, b2h0@p64, b2h1@p0
W2OFF = BVAL + 512  # 4648
SELOFF = W2OFF + OC * HC * 512  # 8740
PINW = SELOFF + BPC * 128  # 9252
AEND = W1OFF + RC * HID // 2  # 2084
BEND = W2OFF

_CACHED_NC = None


def _build_nc():
    import concourse.bass as bass
    import concourse.mybir as mybir
    from concourse.tile import TileContext, add_dep_helper

    f32 = mybir.dt.float32
    f16 = mybir.dt.float16
    relu = mybir.ActivationFunctionType.Relu
    nc = bass.Bass()

    pin = nc.dram_tensor("pin", [128, PINW], f16, kind="ExternalInput")
    out = nc.dram_tensor("out", [BPC, S, OUT], f32, kind="ExternalOutput")

    with TileContext(nc) as tc:
        with (
            tc.tile_pool(name="const", bufs=1) as cpool,
            tc.tile_pool(name="psum_s", bufs=1, space="PSUM") as pp_s,
            tc.tile_pool(name="psum_y", bufs=2, space="PSUM") as pp_y,
            tc.tile_pool(name="psum_t", bufs=2, space="PSUM") as pp_t,
            tc.tile_pool(name="psum_bc", bufs=3, space="PSUM") as pp_bc,
        ):
            p = cpool.tile([128, PINW], f16, tag="pin")
            # 4 input + 4 output DMAs = the 8 DMA semaphore lanes exactly;
            # a 9th DMA would recycle a lane and add a second sync wait on
            # the reusing trigger, which this walrus rejects
            chunks = [0, AEND, BEND, BEND + HC * 512, PINW]
            in_dmas = []
            for i in range(len(chunks) - 1):
                d = nc.sync.dma_start(
                    out=p[:, chunks[i] : chunks[i + 1]],
                    in_=pin[:, chunks[i] : chunks[i + 1]],
                )
                in_dmas.append(d)

            # ---- L1: H[m, h] = x @ W1^T + b1, relu -------------------------
            ph_full = pp_s.tile([128, HID], f32, tag="s")
            ph = ph_full[0:BPC, :]
            for rc in range(RC):
                nc.tensor.matmul(
                    ph[:, :],
                    lhsT=p[:, XTOFF + rc * BPC : XTOFF + (rc + 1) * BPC],
                    rhs=p[:, W1OFF + rc * HID : W1OFF + rc * HID + HID],
                    start=(rc == 0),
                    stop=False,
                )
            nc.tensor.matmul(
                ph[:, :],
                lhsT=p[32:33, I4OFF : I4OFF + BPC],
                rhs=p[32:33, BVAL : BVAL + HID],
                start=False,
                stop=True,
            )
            h_sb = cpool.tile([BPC, HID], f16, tag="h")
            nc.scalar.activation(h_sb[:, :], ph[:, :], relu)

            # ---- H -> H^T (fp16, stationary operand for L2) ----------------
            ht_sb = cpool.tile([128, HC * BPC], f16, tag="ht")
            for hc in range(HC):
                pt = pp_t.tile([128, BPC], f16, tag="t")
                nc.tensor.transpose(
                    pt[:, :],
                    h_sb[0:BPC, hc * 128 : (hc + 1) * 128],
                    p[0:BPC, I4OFF : I4OFF + BPC],
                )
                nc.vector.tensor_copy(ht_sb[:, hc * BPC : (hc + 1) * BPC], pt[:, :])

            # ---- L2: Y[m, o] = H @ W2^T + b2, fp16 y rows ------------------
            y_halves = []
            for oc in range(OC):
                py = pp_y.tile([BPC, 512], f32, tag="y")
                for hc in range(HC):
                    w2c = W2OFF + oc * HC * 512 + hc * 512
                    nc.tensor.matmul(
                        py[:, :],
                        lhsT=ht_sb[:, hc * BPC : (hc + 1) * BPC],
                        rhs=p[:, w2c : w2c + 512],
                        start=(hc == 0),
                        stop=False,
                    )
                bp = 64 if oc == 0 else 0
                ones_c = I4OFF if oc == 0 else BIASOFF
                nc.tensor.matmul(
                    py[:, :],
                    lhsT=p[bp : bp + 1, ones_c : ones_c + BPC],
                    rhs=p[bp : bp + 1, BVAL : BVAL + 512],
                    start=False,
                    stop=True,
                )
                yh = cpool.tile([BPC, 512], f16, tag=f"yh{oc}")
                if oc == 0:
                    nc.vector.tensor_copy(yh[:, :], py[:, :])
                else:
                    nc.scalar.activation(
                        yh[:, :], py[:, :], mybir.ActivationFunctionType.Copy
                    )
                y_halves.append(yh)

            # ---- broadcast rows across partitions, store -------------------
            # A K=4 selector matmul (lhsT = e_b outer ones, host-packed)
            # extracts row b of Y AND replicates it across all 128 output
            # partitions in one PE op. One 4 MiB DMA per batch reads the
            # [128,1024] tile 8x via a stride-0 dim.
            out_dmas = []
            last_act = None
            last_dve = None
            for b in range(BPC):
                ybc = cpool.tile([128, OUT], f32, tag=f"ybc{b}")
                for oc in range(OC):
                    pb = pp_bc.tile([128, 512], f32, tag="bc")
                    last_mm = nc.tensor.matmul(
                        pb[:, :],
                        lhsT=p[0:BPC, SELOFF + b * 128 : SELOFF + (b + 1) * 128],
                        rhs=y_halves[oc][0:BPC, :],
                        start=True,
                        stop=True,
                    )
                    dst = ybc[:, oc * 512 : (oc + 1) * 512]
                    if b % 2 == 0:
                        last_dve = nc.vector.tensor_copy(dst, pb[:, :])
                    else:
                        last_act = nc.scalar.activation(
                            dst, pb[:, :], mybir.ActivationFunctionType.Copy
                        )
                dma_eng = nc.sync if b % 2 == 0 else nc.scalar
                d = dma_eng.dma_start(
                    out=out[b].rearrange("(c p) o -> p c o", c=SC),
                    in_=ybc[:, :].unsqueeze(1).broadcast_to((128, SC, OUT)),
                )
                out_dmas.append(d)

            # The kernel-tail drain waits on every proc's final tick, but this
            # walrus allows at most ONE sync wait per instruction. Chain SP
            # nops, one dependency each, so SP's vector clock observes the
            # final tick of every DMA lane and engine before the drain.
            tail = out_dmas + in_dmas + [last_mm, last_act, last_dve]
            for d in tail:
                n = nc.sync.nop(nofuse=True)
                add_dep_helper(
                    n.ins, d.ins, sync=True, reason="observe final ticks pre-drain"
                )

    return nc


def _get_nc():
    global _CACHED_NC
    if _CACHED_NC is None:
        _CACHED_NC = _build_nc()
    return _CACHED_NC


def _prep_in_maps(representation, W1, b1, W2, b2):
    rep = np.asarray(representation, dtype=np.float32).reshape(B, R)
    w1 = np.asarray(W1, dtype=np.float32)
    w2 = np.asarray(W2, dtype=np.float32)
    b1 = np.asarray(b1, dtype=np.float32)
    b2 = np.asarray(b2, dtype=np.float32)

    # w1p[p, rc*HID + h] = W1[h, rc*128 + p]
    w1p = np.ascontiguousarray(
        w1.T.reshape(RC, 128, HID).transpose(1, 0, 2).reshape(128, RC * HID)
    ).astype(np.float16)
    # w2p[p, oc*HC*512 + hc*512 + j] = W2[oc*512 + j, hc*128 + p]
    w2p = (
        w2.reshape(OC, 512, HC, 128)
        .transpose(3, 0, 2, 1)
        .reshape(128, OC * HC * 512)
    )
    w2p = np.ascontiguousarray(w2p).astype(np.float16)

    in_maps = []
    for c in range(N_CORES):
        xt = rep[c * BPC : (c + 1) * BPC].T  # [R, BPC]
        pin = np.zeros((128, PINW), dtype=np.float16)
        pin[:, XTOFF : XTOFF + RC * BPC] = (
            xt.reshape(RC, 128, BPC).transpose(1, 0, 2).reshape(128, RC * BPC)
        ).astype(np.float16)
        pin[0:BPC, I4OFF : I4OFF + BPC] = np.eye(BPC, dtype=np.float16)
        for q in (32, 64):
            pin[q, I4OFF : I4OFF + BPC] = 1.0
        pin[0, BIASOFF : BIASOFF + BPC] = 1.0
        pin[:, W1OFF : W1OFF + RC * HID] = w1p
        pin[32, BVAL : BVAL + HID] = b1.astype(np.float16)
        pin[64, BVAL : BVAL + 512] = b2[0:512].astype(np.float16)
        pin[0, BVAL : BVAL + 512] = b2[512:1024].astype(np.float16)
        pin[:, W2OFF : W2OFF + OC * HC * 512] = w2p
        for b in range(BPC):
            pin[b, SELOFF + b * 128 : SELOFF + (b + 1) * 128] = 1.0
        in_maps.append({"pin": pin})
    return in_maps


def run_sharded(representation, W1, b1, W2, b2, **run_kwargs):
    """Compile+run on 8 cores; returns (full_output, BassKernelResults)."""
    from concourse.bass_utils import run_bass_kernel_spmd

    nc = _get_nc()
    in_maps = _prep_in_maps(representation, W1, b1, W2, b2)
    res = run_bass_kernel_spmd(nc, in_maps, core_ids=list(range(N_CORES)), **run_kwargs)
    full = np.concatenate([r["out"] for r in res.results], axis=0)
    return full, res


def kernel(representation, size_matrix=None, W1=None, b1=None, W2=None, b2=None):
    # size_matrix only contributes its shape in the reference (ones_like);
    # its values are unused.
    full, _ = run_sharded(representation, W1, b1, W2, b2)
    return full


# revision 12
# speedup vs baseline: 1.4670x; 1.0479x over previous
"""Trainium2 Bass kernel for nn_Decoder_14894946583396 (dense_mlp).

Reference computation:
    sized = broadcast(representation[B,1,R] -> [B,S,R])   (ones @ rep)
    h     = relu(sized @ W1^T + b1)                       [B,S,HID]
    out   = h @ W2^T + b2                                 [B,S,OUT]

Because every position s within batch b receives the identical input row
representation[b], the MLP output row is identical for all S positions:
    row[b] = relu(rep[b] @ W1^T + b1) @ W2^T + b2         [B,OUT]
    out[b, s, :] = row[b]  for all s

Data-parallel across 8 NeuronCores: 4 batches per core, replicated
weights. The per-core kernel computes the tiny MLP in fp16 on the
TensorEngine (1 cycle/row vs fp32's 4) and broadcast-writes each row
across S with stride-0-source SBUF->DRAM DMAs.

Device pipeline per core:
  1. ALL inputs live in one packed fp16 DRAM tensor, streamed over the
     sync HWDGE ring as 5 chunked DMAs in consumption order:
       A: x^T + I4 + ones rows + W1h0   (so L1 starts on arrival)
       B: W1h1 + bias block (b1@p32, b2h0@p64, b2h1@p96)
       C0/C1: W2 per output half
       D: selector blocks
     A single ring avoids the cross-queue SDMA thrash that halved
     aggregate read bandwidth when inputs were spread over 3 queues.
  2. No warmup: L1's own matmul stream is the HAM-ramp activity.
  3. L1: H[m,h] = x @ W1^T via 8 accumulating fp16 matmuls (x^T chunk
     stationary) + a K=1 ones-matmul folding in b1; relu casts to fp16
     on ScalarE.
  4. H -> H^T via 4 fp16 PE transposes, copied to SBUF on DVE.
  5. L2 per 512-col half: Y = H @ W2^T + b2 (5 fp16 matmuls + K=1 bias
     matmul vs ones@p64/p96); PSUM -> SBUF fp16 cast (DVE half 0, ACT
     half 1).
  6. Broadcast per batch/half: K=4 selector matmul replicates row b
     across 128 partitions; PSUM -> SBUF f32 copy (DVE even b, ACT
     odd b).
  7. One 4 MiB DMA per batch: out[b] viewed [p=128, c=8, o=1024] fed
     from the [128,1024] broadcast tile with a stride-0 c dim (each
     partition's 4 KB row is read 8x). Even b on the sync ring, odd b
     on the scalar ring.

The Tile layer auto-inserts single-wait sync NOPs where an instruction
would need 2+ semaphore waits; the explicit nop chain before the
TileContext exit keeps the final drain itself at <=1 wait.
"""

import sys

import numpy as np

if "/opt/trn_rl_repo" not in sys.path:
    sys.path.insert(0, "/opt/trn_rl_repo")

B, S, R = 32, 1024, 1024
HID, OUT = 512, 1024
N_CORES = 8
BPC = B // N_CORES  # batches per core

RC = R // 128  # layer-1 contraction chunks
HC = HID // 128  # layer-2 contraction chunks
OC = OUT // 512  # 512-wide output column chunks
SC = S // 128  # broadcast repeats per output DMA

# pin columns (fp16), in DMA-chunk order:
#   A: x^T (32) | I4+ones (4) | W1h0 (2048)
#   B: W1h1 (2048) | bias block (512: b1@p32, b2h0@p64, b2h1@p96)
#   C: W2 oc0 (2048) | W2 oc1 (2048)
#   D: selector blocks (512, rows 0..3)
XTOFF = 0
I4OFF = XTOFF + RC * BPC  # 32
W1OFF = I4OFF + BPC  # 36
BIASOFF = W1OFF + RC * HID  # 4132, first 4 cols: ones row at p0
BVAL = BIASOFF + 4  # bias values: b1@p32, b2h0@p64, b2h1@p0
W2OFF = BVAL + 512  # 4648
SELOFF = W2OFF + OC * HC * 512  # 8740
PINW = SELOFF + BPC * 128  # 9252
AEND = W1OFF + RC * HID // 2  # 2084
BEND = W2OFF

_CACHED_NC = None


def _build_nc():
    import concourse.bass as bass
    import concourse.mybir as mybir
    from concourse.tile import TileContext, add_dep_helper

    f32 = mybir.dt.float32
    f16 = mybir.dt.float16
    relu = mybir.ActivationFunctionType.Relu
    nc = bass.Bass()

    pin = nc.dram_tensor("pin", [128, PINW], f16, kind="ExternalInput")
    out = nc.dram_tensor("out", [BPC, S, OUT], f32, kind="ExternalOutput")

    with TileContext(nc) as tc:
        with (
            tc.tile_pool(name="const", bufs=1) as cpool,
            tc.tile_pool(name="psum_s", bufs=1, space="PSUM") as pp_s,
            tc.tile_pool(name="psum_y", bufs=2, space="PSUM") as pp_y,
            tc.tile_pool(name="psum_t", bufs=2, space="PSUM") as pp_t,
            tc.tile_pool(name="psum_bc", bufs=3, space="PSUM") as pp_bc,
        ):
            p = cpool.tile([128, PINW], f16, tag="pin")
            # 4 input + 4 output DMAs = the 8 DMA semaphore lanes exactly;
            # a 9th DMA would recycle a lane and add a second sync wait on
            # the reusing trigger, which this walrus rejects
            chunks = [0, AEND, BEND, BEND + HC * 512, PINW]
            in_dmas = []
            for i in range(len(chunks) - 1):
                d = nc.sync.dma_start(
                    out=p[:, chunks[i] : chunks[i + 1]],
                    in_=pin[:, chunks[i] : chunks[i + 1]],
                )
                in_dmas.append(d)

            # ---- PE warmup: two fp32 matmuls on zeros (4 cycles/row = high
            # sustained activity) force the HAM clock ramp (1.2 -> 2.4 GHz)
            # during the otherwise-idle input-DMA window; fp16 L1 matmuls
            # alone never trip the ramp threshold ------------------------
            wm_sb = cpool.tile([128, 512], f32, tag="wm")
            nc.vector.memset(wm_sb[:, :], 0.0)
            ph_full = pp_s.tile([128, HID], f32, tag="s")
            for k in range(2):
                nc.tensor.matmul(
                    ph_full[:, :],
                    lhsT=wm_sb[:, 0:128],
                    rhs=wm_sb[:, :],
                    start=True,
                    stop=True,
                )

            # ---- L1: H[m, h] = x @ W1^T + b1, relu -------------------------
            ph = ph_full[0:BPC, :]
            for rc in range(RC):
                nc.tensor.matmul(
                    ph[:, :],
                    lhsT=p[:, XTOFF + rc * BPC : XTOFF + (rc + 1) * BPC],
                    rhs=p[:, W1OFF + rc * HID : W1OFF + rc * HID + HID],
                    start=(rc == 0),
                    stop=False,
                )
            nc.tensor.matmul(
                ph[:, :],
                lhsT=p[32:33, I4OFF : I4OFF + BPC],
                rhs=p[32:33, BVAL : BVAL + HID],
                start=False,
                stop=True,
            )
            h_sb = cpool.tile([BPC, HID], f16, tag="h")
            nc.scalar.activation(h_sb[:, :], ph[:, :], relu)

            # ---- H -> H^T (fp16, stationary operand for L2) ----------------
            ht_sb = cpool.tile([128, HC * BPC], f16, tag="ht")
            for hc in range(HC):
                pt = pp_t.tile([128, BPC], f16, tag="t")
                nc.tensor.transpose(
                    pt[:, :],
                    h_sb[0:BPC, hc * 128 : (hc + 1) * 128],
                    p[0:BPC, I4OFF : I4OFF + BPC],
                )
                nc.vector.tensor_copy(ht_sb[:, hc * BPC : (hc + 1) * BPC], pt[:, :])

            # ---- L2: Y[m, o] = H @ W2^T + b2, fp16 y rows ------------------
            y_halves = []
            for oc in range(OC):
                py = pp_y.tile([BPC, 512], f32, tag="y")
                for hc in range(HC):
                    w2c = W2OFF + oc * HC * 512 + hc * 512
                    nc.tensor.matmul(
                        py[:, :],
                        lhsT=ht_sb[:, hc * BPC : (hc + 1) * BPC],
                        rhs=p[:, w2c : w2c + 512],
                        start=(hc == 0),
                        stop=False,
                    )
                bp = 64 if oc == 0 else 0
                ones_c = I4OFF if oc == 0 else BIASOFF
                nc.tensor.matmul(
                    py[:, :],
                    lhsT=p[bp : bp + 1, ones_c : ones_c + BPC],
                    rhs=p[bp : bp + 1, BVAL : BVAL + 512],
                    start=False,
                    stop=True,
                )
                yh = cpool.tile([BPC, 512], f16, tag=f"yh{oc}")
                if oc == 0:
                    nc.vector.tensor_copy(yh[:, :], py[:, :])
                else:
                    nc.scalar.activation(
                        yh[:, :], py[:, :], mybir.ActivationFunctionType.Copy
                    )
                y_halves.append(yh)

            # ---- broadcast rows across partitions, store -------------------
            # A K=4 selector matmul (lhsT = e_b outer ones, host-packed)
            # extracts row b of Y AND replicates it across all 128 output
            # partitions in one PE op. One 4 MiB DMA per batch reads the
            # [128,1024] tile 8x via a stride-0 dim.
            out_dmas = []
            last_act = None
            last_dve = None
            for b in range(BPC):
                ybc = cpool.tile([128, OUT], f32, tag=f"ybc{b}")
                for oc in range(OC):
                    pb = pp_bc.tile([128, 512], f32, tag="bc")
                    last_mm = nc.tensor.matmul(
                        pb[:, :],
                        lhsT=p[0:BPC, SELOFF + b * 128 : SELOFF + (b + 1) * 128],
                        rhs=y_halves[oc][0:BPC, :],
                        start=True,
                        stop=True,
                    )
                    dst = ybc[:, oc * 512 : (oc + 1) * 512]
                    if b % 2 == 0:
                        last_dve = nc.vector.tensor_copy(dst, pb[:, :])
                    else:
                        last_act = nc.scalar.activation(
                            dst, pb[:, :], mybir.ActivationFunctionType.Copy
                        )
                dma_eng = nc.sync if b % 2 == 0 else nc.scalar
                d = dma_eng.dma_start(
                    out=out[b].rearrange("(c p) o -> p c o", c=SC),
                    in_=ybc[:, :].unsqueeze(1).broadcast_to((128, SC, OUT)),
                )
                out_dmas.append(d)

            # The kernel-tail drain waits on every proc's final tick, but this
            # walrus allows at most ONE sync wait per instruction. Chain SP
            # nops, one dependency each, so SP's vector clock observes the
            # final tick of every DMA lane and engine before the drain.
            tail = out_dmas + in_dmas + [last_mm, last_act, last_dve]
            for d in tail:
                n = nc.sync.nop(nofuse=True)
                add_dep_helper(
                    n.ins, d.ins, sync=True, reason="observe final ticks pre-drain"
                )

    return nc


def _get_nc():
    global _CACHED_NC
    if _CACHED_NC is None:
        _CACHED_NC = _build_nc()
    return _CACHED_NC


def _prep_in_maps(representation, W1, b1, W2, b2):
    rep = np.asarray(representation, dtype=np.float32).reshape(B, R)
    w1 = np.asarray(W1, dtype=np.float32)
    w2 = np.asarray(W2, dtype=np.float32)
    b1 = np.asarray(b1, dtype=np.float32)
    b2 = np.asarray(b2, dtype=np.float32)

    # w1p[p, rc*HID + h] = W1[h, rc*128 + p]
    w1p = np.ascontiguousarray(
        w1.T.reshape(RC, 128, HID).transpose(1, 0, 2).reshape(128, RC * HID)
    ).astype(np.float16)
    # w2p[p, oc*HC*512 + hc*512 + j] = W2[oc*512 + j, hc*128 + p]
    w2p = (
        w2.reshape(OC, 512, HC, 128)
        .transpose(3, 0, 2, 1)
        .reshape(128, OC * HC * 512)
    )
    w2p = np.ascontiguousarray(w2p).astype(np.float16)

    in_maps = []
    for c in range(N_CORES):
        xt = rep[c * BPC : (c + 1) * BPC].T  # [R, BPC]
        pin = np.zeros((128, PINW), dtype=np.float16)
        pin[:, XTOFF : XTOFF + RC * BPC] = (
            xt.reshape(RC, 128, BPC).transpose(1, 0, 2).reshape(128, RC * BPC)
        ).astype(np.float16)
        pin[0:BPC, I4OFF : I4OFF + BPC] = np.eye(BPC, dtype=np.float16)
        for q in (32, 64):
            pin[q, I4OFF : I4OFF + BPC] = 1.0
        pin[0, BIASOFF : BIASOFF + BPC] = 1.0
        pin[:, W1OFF : W1OFF + RC * HID] = w1p
        pin[32, BVAL : BVAL + HID] = b1.astype(np.float16)
        pin[64, BVAL : BVAL + 512] = b2[0:512].astype(np.float16)
        pin[0, BVAL : BVAL + 512] = b2[512:1024].astype(np.float16)
        pin[:, W2OFF : W2OFF + OC * HC * 512] = w2p
        for b in range(BPC):
            pin[b, SELOFF + b * 128 : SELOFF + (b + 1) * 128] = 1.0
        in_maps.append({"pin": pin})
    return in_maps


def run_sharded(representation, W1, b1, W2, b2, **run_kwargs):
    """Compile+run on 8 cores; returns (full_output, BassKernelResults)."""
    from concourse.bass_utils import run_bass_kernel_spmd

    nc = _get_nc()
    in_maps = _prep_in_maps(representation, W1, b1, W2, b2)
    res = run_bass_kernel_spmd(nc, in_maps, core_ids=list(range(N_CORES)), **run_kwargs)
    full = np.concatenate([r["out"] for r in res.results], axis=0)
    return full, res


def kernel(representation, size_matrix=None, W1=None, b1=None, W2=None, b2=None):
    # size_matrix only contributes its shape in the reference (ones_like);
    # its values are unused.
    full, _ = run_sharded(representation, W1, b1, W2, b2)
    return full
